# revision 1
# baseline (speedup 1.0000x reference)
"""Trainium2 Bass kernel for nn_MCUDetectionLoss.

Strategy (data-parallel over batch, 8 cores, B=16 -> 2 images/core):

The loss only touches (a) the objectness channel cls_p[:, 0] in full and
(b) 32 gathered cells per image (63-class column + 4 reg values).  The host
therefore ships each core:
  - obj   [128, 320]  objectness maps (scale3 flat 32768 = cols 0:256,
                      scale4 flat 8192 = cols 256:320)
  - tblc  [40960, 64] cls maps transposed to cell-major (gather table)
  - tblr  [40960, 4]  reg maps transposed to cell-major
  - per-target metadata (raw t rows, cell indices, floor(gx/gy), scale)

Device program per core: two indirect-DMA gathers (cell rows), softplus of
the obj map via exp/ln (one ACT table set), per-target smooth-L1 box loss,
positive-BCE, focal loss, duplicate-cell counts via an equality matrix, and
a single matmul against partition masks reducing everything to a [2, 8]
partials vector.  The host combines the 8 partials vectors into the scalar.

Identities used (bce = BCEWithLogits):
  bce(x, 0) = softplus(x);  bce(x, 1) = softplus(x) - x
  bce(x, y) = softplus(x) - x*y for y in {0,1}
  focal (1-pt)^2 = (p-y)^2 = ((1/(1+e^x)) + y - 1)^2
  sum softplus(obj)*bg = sum_all softplus - sum_targets softplus(obj_t)/count_t
where count_t = number of targets sharing the same (image, cell) -- computed
on-device from an equality matrix of cell ids (exact for duplicates).
"""

import sys

for _p in ("/opt/trn_rl_repo", "/root/.axon_site/_ro/trn_rl_repo"):
    if _p not in sys.path:
        sys.path.append(_p)

import numpy as np

import concourse.bass as bass
import concourse.tile as tile
from concourse import mybir
from concourse.bass_utils import run_bass_kernel_spmd

AF = mybir.ActivationFunctionType
ALU = mybir.AluOpType
AX = mybir.AxisListType
F32 = mybir.dt.float32
I32 = mybir.dt.int32

ALPHA = 0.25
BBOX_W, OBJ_W, CLS_W = 2.0, 1.0, 0.5

M = 8          # cores
B, T, NC_CLS = 16, 32, 63
H3 = W3 = 128
H4 = W4 = 64
BL = B // M    # images per core
N3 = BL * H3 * W3   # 32768 scale3 cells per core
N4 = BL * H4 * W4   # 8192 scale4 cells per core
NTOT = N3 + N4      # 40960 table rows per core
NT = 2 * BL * T     # 128 targets per core (64 scale3 + 64 scale4)

_NC_CACHE = None


def _build_bass():
    nc = bass.Bass("TRN2", target_bir_lowering=False, debug=False, num_devices=M)
    obj = nc.declare_dram_parameter("obj", [128, NTOT // 128], F32, isOutput=False)
    tblcr = nc.declare_dram_parameter("tblcr", [NTOT, 68], F32, isOutput=False)
    # meta cols: 0:5 tgt row, 5 cellidx(f32), 6:8 gx gy, 8 w-scale,
    # 9 cellidx int32 (bit pattern)
    meta = nc.declare_dram_parameter("meta", [NT, 10], F32, isOutput=False)
    crow = nc.declare_dram_parameter("crow", [NT, NT], F32, isOutput=False)
    part = nc.declare_dram_parameter("part", [2, 7], F32, isOutput=True)

    objw = NTOT // 128          # 320
    c3 = N3 // 128              # 256 cols of scale3 cells
    CC = NC_CLS                 # 63
    H = NT // 2                 # 64: gather split point

    from contextlib import ExitStack
    with ExitStack() as st:
        def sb(name, shape, dt=F32):
            return st.enter_context(nc.sbuf_tensor(name, shape, dt))

        obj_t = sb("obj_t", [128, objw]); meta_t = sb("meta_t", [NT, 10])
        crow_t = sb("crow_t", [NT, NT])
        gcomb = sb("gcomb", [NT, 68])      # 0 obj, 1:64 cls, 64:68 reg
        stats = sb("stats", [NT, 7]); cbias = sb("cbias", [128, 1])
        masks = sb("masks_t", [NT, 2]); warm = sb("warm", [128, 1])
        e_t = sb("e_t", [128, objw]); sp_t = sb("sp_t", [128, objw])
        e_mat = sb("e_mat", [NT, NT]); cnt = sb("cnt", [NT, 1])
        lcnt = sb("lcnt", [NT, 1]); rec = sb("rec", [NT, 1])
        e_comb = sb("e_comb", [NT, 66]); scl = sb("scl", [NT, 66])
        rxf = sb("rxf", [NT, 66])          # exp(-scl); 1:64 = 1-p, 64:66 = sig
        txywh = sb("txywh", [NT, 4]); clp = sb("clp", [NT, 2])
        dwh = sb("dwh", [NT, 2]); g2 = sb("g2", [NT, 2])
        a_t = sb("a_t", [NT, 2]); c_t = sb("c_t", [NT, 2])
        dt_ = sb("dt_", [NT, 4]); dabs = sb("dabs", [NT, 4])
        mt = sb("mt", [NT, 4]); msq = sb("msq", [NT, 4]); dm = sb("dm", [NT, 4])
        sl1 = sb("sl1", [NT, 4]); sl1s = sb("sl1s", [NT, 4])
        iot = sb("iot", [NT, CC], I32); iotf = sb("iotf", [NT, CC])
        y_t = sb("y_t", [NT, CC]); xy = sb("xy", [NT, CC])
        bce = sb("bce", [NT, CC]); u1 = sb("u1", [NT, CC])
        q2 = sb("q2", [NT, CC]); fq = sb("fq", [NT, CC])
        out_t = sb("out_t", [2, 7])
        pout = st.enter_context(nc.psum_tensor("pout", [2, 7], F32))

        meta_sem = st.enter_context(nc.semaphore("meta_sem"))
        crow_sem = st.enter_context(nc.semaphore("crow_sem"))
        obj_sem = st.enter_context(nc.semaphore("obj_sem"))
        gat_sem = st.enter_context(nc.semaphore("gat_sem"))
        gp_sem = st.enter_context(nc.semaphore("gp_sem"))
        act_sem = st.enter_context(nc.semaphore("act_sem"))
        dve_sem = st.enter_context(nc.semaphore("dve_sem"))
        pe_sem = st.enter_context(nc.semaphore("pe_sem"))
        st_sem = st.enter_context(nc.semaphore("st_sem"))
        block = st.enter_context(nc.Block())

        one_b = cbias[:, 0:1]

        # ACT landmarks (indices into the scalar stream below)
        A_SCL, A_RXSIG, A_DWH, A_REC, A_SL1S, A_OUT = 3, 4, 5, 10, 11, 12
        # DVE landmarks
        D_CNT, D_CLP, D_SL1, D_FQ = 2, 6, 22, 23

        @block.sync
        def _(sync):
            sync.dma_start(out=meta_t[:], in_=meta[:]).then_inc(meta_sem, 16)
            sync.dma_start(out=crow_t[:], in_=crow[:]).then_inc(crow_sem, 16)
            sync.dma_start(out=obj_t[:], in_=obj[:]).then_inc(obj_sem, 16)
            sync.wait_ge(act_sem, A_OUT)
            sync.dma_start(out=part[:], in_=out_t[:]).then_inc(st_sem, 16)

        @block.gpsimd
        def _(gpsimd):
            gpsimd.memset(cbias[:], 1.0).then_inc(gp_sem, 1)           # 1
            gpsimd.memset(masks[:], 0.0).then_inc(gp_sem, 1)           # 2
            gpsimd.memset(masks[0:64, 0:1], 1.0).then_inc(gp_sem, 1)   # 3
            gpsimd.memset(masks[64:128, 1:2], 1.0).then_inc(gp_sem, 1)  # 4
            gpsimd.iota(out=iot[:], pattern=[[1, CC]], base=0,
                        channel_multiplier=0).then_inc(gp_sem, 1)      # 5
            gpsimd.drain()
            gpsimd.tensor_copy(out=iotf[:], in_=iot[:]).then_inc(gp_sem, 1)  # 6
            gpsimd.wait_ge(meta_sem, 16)
            gpsimd.indirect_dma_start(
                out=gcomb[:], out_offset=None, in_=tblcr[:],
                in_offset=bass.IndirectOffsetOnAxis(
                    ap=meta_t[:, 9:10].bitcast(I32), axis=0),
            ).then_inc(gat_sem, 16)

        @block.scalar
        def _(scalar):
            A = AF
            act = nc.scalar
            # warmup: triggers the ACT table load before any data is ready
            act.activation(out=warm[:], in_=warm[:],
                           func=A.Exp).then_inc(act_sem, 1)             # 1
            scalar.wait_ge(gat_sem, 16)
            scalar.wait_ge(gp_sem, 1)
            act.activation(out=e_comb[:], in_=gcomb[:, 0:66],
                           func=A.Exp).then_inc(act_sem, 1)             # 2
            act.activation(out=scl[:], in_=e_comb[:], func=A.Ln,
                           bias=one_b).then_inc(act_sem, 1)             # 3 A_SCL
            act.activation(out=rxf[:], in_=scl[:], func=A.Exp,
                           scale=-1.0).then_inc(act_sem, 1)             # 4 A_RXSIG
            scalar.wait_ge(dve_sem, D_CLP)
            act.activation(out=dwh[:], in_=clp[:],
                           func=A.Exp).then_inc(act_sem, 1)             # 5 A_DWH
            scalar.wait_ge(obj_sem, 16)
            act.activation(out=e_t[:], in_=obj_t[:],
                           func=A.Exp).then_inc(act_sem, 1)             # 6
            act.activation(out=sp_t[:, 0:c3], in_=e_t[:, 0:c3], func=A.Ln,
                           bias=one_b,
                           accum_out=stats[:, 5:6]).then_inc(act_sem, 1)  # 7
            act.activation(out=sp_t[:, c3:objw], in_=e_t[:, c3:objw],
                           func=A.Ln, bias=one_b,
                           accum_out=stats[:, 6:7]).then_inc(act_sem, 1)  # 8
            scalar.wait_ge(dve_sem, D_CNT)
            act.activation(out=lcnt[:], in_=cnt[:],
                           func=A.Ln).then_inc(act_sem, 1)              # 9
            act.activation(out=rec[:], in_=lcnt[:], func=A.Exp,
                           scale=-1.0).then_inc(act_sem, 1)             # 10 A_REC
            scalar.wait_ge(dve_sem, D_SL1)
            act.activation(out=sl1s[:], in_=sl1[:], func=A.Copy, scale=0.25,
                           accum_out=stats[:, 0:1]).then_inc(act_sem, 1)  # 11 A_SL1S
            scalar.wait_ge(pe_sem, 1)
            act.activation(out=out_t[:], in_=pout[:],
                           func=A.Copy).then_inc(act_sem, 1)            # 12 A_OUT

        @block.vector
        def _(vector):
            vec = nc.vector
            vector.wait_ge(meta_sem, 16)
            vector.wait_ge(crow_sem, 16)
            tgt_c = meta_t[:, 0:5]
            cxf_c = meta_t[:, 5:6]
            gxy_c = meta_t[:, 6:8]
            whs_c = meta_t[:, 8:9]
            spo = scl[:, 0:1]
            spx = scl[:, 1:64]
            rx = rxf[:, 1:64]
            sig = rxf[:, 64:66]
            # groups of mutually-independent ops, one pipe drain per boundary
            vec.tensor_scalar(out=e_mat[:], in0=crow_t[:], scalar1=cxf_c,
                              scalar2=None,
                              op0=ALU.is_equal).then_inc(dve_sem, 1)    # 1
            nc.vector.drain()
            vec.reduce_sum(out=cnt[:], in_=e_mat[:],
                           axis=AX.X).then_inc(dve_sem, 1)              # 2 D_CNT
            vec.tensor_scalar_mul(out=txywh[:], in0=tgt_c[:, 1:5],
                                  scalar1=whs_c).then_inc(dve_sem, 1)   # 3
            nc.vector.drain()
            vec.tensor_tensor(out=g2[:], in0=gxy_c, in1=txywh[:, 0:2],
                              op=ALU.subtract).then_inc(dve_sem, 1)     # 4
            vector.wait_ge(gp_sem, 6)
            vec.tensor_scalar(out=y_t[:], in0=iotf[:], scalar1=tgt_c[:, 0:1],
                              scalar2=None,
                              op0=ALU.is_equal).then_inc(dve_sem, 1)    # 5
            nc.vector.drain()
            vector.wait_ge(gat_sem, 16)
            vec.tensor_scalar(out=clp[:], in0=gcomb[:, 66:68], scalar1=-4.0,
                              scalar2=4.0, op0=ALU.max,
                              op1=ALU.min).then_inc(dve_sem, 1)         # 6 D_CLP
            vec.tensor_tensor(out=xy[:], in0=gcomb[:, 1:64], in1=y_t[:],
                              op=ALU.mult).then_inc(dve_sem, 1)         # 7
            vector.wait_ge(act_sem, A_RXSIG)
            vec.tensor_tensor(out=a_t[:], in0=sig, in1=g2[:],
                              op=ALU.add).then_inc(dve_sem, 1)          # 8
            vector.wait_ge(act_sem, A_DWH)
            vec.tensor_tensor(out=c_t[:], in0=dwh[:], in1=txywh[:, 2:4],
                              op=ALU.subtract).then_inc(dve_sem, 1)     # 9
            nc.vector.drain()
            vec.scalar_tensor_tensor(out=dt_[:, 0:2], in0=c_t[:], scalar=-0.5,
                                     in1=a_t[:], op0=ALU.mult,
                                     op1=ALU.add).then_inc(dve_sem, 1)  # 10
            vec.scalar_tensor_tensor(out=dt_[:, 2:4], in0=c_t[:], scalar=0.5,
                                     in1=a_t[:], op0=ALU.mult,
                                     op1=ALU.add).then_inc(dve_sem, 1)  # 11
            vec.tensor_tensor(out=bce[:], in0=spx, in1=xy[:],
                              op=ALU.subtract).then_inc(dve_sem, 1)     # 12
            nc.vector.drain()
            vec.scalar_tensor_tensor(out=dabs[:], in0=dt_[:], scalar=-1.0,
                                     in1=dt_[:], op0=ALU.mult,
                                     op1=ALU.max).then_inc(dve_sem, 1)  # 13
            vec.scalar_tensor_tensor(out=u1[:], in0=rx, scalar=-1.0,
                                     in1=y_t[:], op0=ALU.add,
                                     op1=ALU.add).then_inc(dve_sem, 1)  # 14
            nc.vector.drain()
            vec.tensor_scalar_min(out=mt[:], in0=dabs[:],
                                  scalar1=1.0).then_inc(dve_sem, 1)     # 15
            vec.tensor_tensor(out=q2[:], in0=u1[:], in1=u1[:],
                              op=ALU.mult).then_inc(dve_sem, 1)         # 16
            nc.vector.drain()
            vec.scalar_tensor_tensor(out=dm[:], in0=mt[:], scalar=-1.0,
                                     in1=dabs[:], op0=ALU.mult,
                                     op1=ALU.add).then_inc(dve_sem, 1)  # 17
            vec.tensor_tensor(out=msq[:], in0=mt[:], in1=mt[:],
                              op=ALU.mult).then_inc(dve_sem, 1)         # 18
            vec.tensor_tensor(out=stats[:, 1:2], in0=spo,
                              in1=gcomb[:, 0:1],
                              op=ALU.subtract).then_inc(dve_sem, 1)     # 19
            vector.wait_ge(act_sem, A_REC)
            vec.tensor_copy(out=stats[:, 4:5],
                            in_=rec[:]).then_inc(dve_sem, 1)            # 20
            vec.tensor_tensor(out=stats[:, 3:4], in0=spo, in1=rec[:],
                              op=ALU.mult).then_inc(dve_sem, 1)         # 21
            nc.vector.drain()
            vec.scalar_tensor_tensor(out=sl1[:], in0=msq[:], scalar=0.5,
                                     in1=dm[:], op0=ALU.mult,
                                     op1=ALU.add).then_inc(dve_sem, 1)  # 22 D_SL1
            vec.scalar_tensor_tensor(out=fq[:], in0=q2[:],
                                     scalar=ALPHA / CC, in1=bce[:],
                                     op0=ALU.mult, op1=ALU.mult,
                                     accum_out=stats[:, 2:3],
                                     ).then_inc(dve_sem, 1)             # 23 D_FQ

        @block.tensor
        def _(tensor):
            tensor.wait_ge(gp_sem, 4)
            tensor.wait_ge(act_sem, A_SL1S)
            tensor.wait_ge(dve_sem, D_FQ)
            nc.tensor.matmul(out=pout[:], lhsT=masks[:], rhs=stats[:],
                             start=True, stop=True).then_inc(pe_sem, 1)

    return nc


def _get_bass():
    global _NC_CACHE
    if _NC_CACHE is None:
        _NC_CACHE = _build_bass()
    return _NC_CACHE


def _prep_core_inputs(cls_p3, reg_p3, cls_p4, reg_p4, t3, t4):
    """Slice/transpose full inputs into the 8 per-core input maps."""
    f = np.float32
    obj3 = np.ascontiguousarray(cls_p3[:, 0]).reshape(M, N3)
    obj4 = np.ascontiguousarray(cls_p4[:, 0]).reshape(M, N4)
    a3 = np.ascontiguousarray(cls_p3.transpose(0, 2, 3, 1)).reshape(M, N3, 64)
    a4 = np.ascontiguousarray(cls_p4.transpose(0, 2, 3, 1)).reshape(M, N4, 64)
    r3 = np.ascontiguousarray(reg_p3.transpose(0, 2, 3, 1)).reshape(M, N3, 4)
    r4 = np.ascontiguousarray(reg_p4.transpose(0, 2, 3, 1)).reshape(M, N4, 4)

    in_maps = []
    for c in range(M):
        sl = slice(c * BL, (c + 1) * BL)
        lt3, lt4 = t3[sl], t4[sl]
        obj = np.concatenate([obj3[c], obj4[c]]).reshape(128, NTOT // 128)
        tblcr = np.concatenate([
            np.concatenate([a3[c], r3[c]], axis=1),
            np.concatenate([a4[c], r4[c]], axis=1)])
        tblcr[:, 64:66] *= -1.0   # so one exp() pass covers sigmoid inputs
        tgt = np.concatenate(
            [lt3.reshape(-1, 5), lt4.reshape(-1, 5)]).astype(f)

        cellidx = np.zeros(NT, np.int32)
        gxy = np.zeros((NT, 2), f)
        whs = np.zeros((NT, 1), f)
        for s, (lt, hh, ww, base, stride) in enumerate(
                [(lt3, H3, W3, 0, H3 * W3), (lt4, H4, W4, N3, H4 * W4)]):
            tx = lt[..., 1] * ww
            ty = lt[..., 2] * hh
            gx = np.clip(tx, 0, ww - 1).astype(np.int32)
            gy = np.clip(ty, 0, hh - 1).astype(np.int32)
            bb = np.arange(BL)[:, None]
            rows = slice(s * BL * T, (s + 1) * BL * T)
            cellidx[rows] = (base + bb * stride + gy * ww + gx).reshape(-1)
            gxy[rows, 0] = gx.reshape(-1)
            gxy[rows, 1] = gy.reshape(-1)
            whs[rows, 0] = ww
        meta = np.zeros((NT, 10), f)
        meta[:, 0:5] = tgt
        meta[:, 5] = cellidx
        meta[:, 6:8] = gxy
        meta[:, 8:9] = whs
        meta[:, 9] = cellidx.view(f)          # int32 bits for indirect DMA
        crow = np.broadcast_to(
            cellidx[None, :].astype(f), (NT, NT)).copy()
        in_maps.append({
            "obj": np.ascontiguousarray(obj, f),
            "tblcr": np.ascontiguousarray(tblcr, f),
            "meta": meta,
            "crow": crow,
        })
    return in_maps


def _combine(parts):
    """parts: [8, 2, 8] per-core partials -> scalar loss (float64 combine)."""
    P = np.asarray(parts, np.float64)
    lb3, lb4 = P[:, 0, 0].sum(), P[:, 1, 0].sum()
    lo3p, lo4p = P[:, 0, 1].sum(), P[:, 1, 1].sum()
    lc3, lc4 = P[:, 0, 2].sum(), P[:, 1, 2].sum()
    corr3, corr4 = P[:, 0, 3].sum(), P[:, 1, 3].sum()
    uniq3, uniq4 = P[:, 0, 4].sum(), P[:, 1, 4].sum()
    sall3 = P[:, 0, 5].sum() + P[:, 1, 5].sum()
    sall4 = P[:, 0, 6].sum() + P[:, 1, 6].sum()

    bg3 = (sall3 - corr3) / max(B * H3 * W3 - uniq3, 1.0)
    bg4 = (sall4 - corr4) / max(B * H4 * W4 - uniq4, 1.0)
    lo3 = lo3p + 0.05 * bg3
    lo4 = lo4p + 0.05 * bg4
    n = 2 * B * T
    lb = (lb3 + lb4) / n
    lc = (lc3 + lc4) / n
    lo = (lo3 + lo4) / max(n, 1)
    return np.float32(BBOX_W * lb + OBJ_W * lo + CLS_W * lc)


def kernel(cls_p3, reg_p3, cls_p4, reg_p4, t3, t4, _trace=False):
    in_maps = _prep_core_inputs(
        np.asarray(cls_p3), np.asarray(reg_p3), np.asarray(cls_p4),
        np.asarray(reg_p4), np.asarray(t3), np.asarray(t4))
    nc = _get_bass()
    res = run_bass_kernel_spmd(nc, in_maps, core_ids=list(range(M)),
                               trace=_trace)
    parts = np.stack([r["part"] for r in res.results])
    out = _combine(parts)
    if _trace:
        return out, res
    return out


if __name__ == "__main__":
    rng = np.random.default_rng(0)
    inputs = {
        "cls_p3": rng.standard_normal((B, 64, H3, W3), np.float32),
        "reg_p3": rng.standard_normal((B, 4, H3, W3), np.float32),
        "cls_p4": rng.standard_normal((B, 64, H4, W4), np.float32),
        "reg_p4": rng.standard_normal((B, 4, H4, W4), np.float32),
        "t3": rng.random((B, T, 5), np.float32),
        "t4": rng.random((B, T, 5), np.float32),
    }
    print(kernel(**inputs))



# revision 2
# speedup vs baseline: 1.3770x; 1.3770x over previous
"""Trainium2 Bass kernel for nn_MCUDetectionLoss.

Strategy (data-parallel over batch, 8 cores, B=16 -> 2 images/core):

The loss touches (a) the objectness channel cls_p[:, 0] in full and (b) 64+64
gathered cells per core (obj/cls/reg values at target cells).  The host slices
each core's two images, gathers the 128 target rows (cheap fancy indexing),
and ships two small tensors per core:
  - gath [128, 200]  per-target row: aux cols, one-hot products, and the
                     66-col activation block [obj, cls63, -r0, -r1] plus
                     -clip(r2..r3) so a single Exp/Ln/Exp chain yields
                     softplus, 1-p, sigmoid and exp(clip) in one pass
  - objd [128, 320]  objectness maps (scale3 flat 32768 = cols 0:256,
                     scale4 flat 8192 = cols 256:320)

Device program per core: two parallel input DMAs (sync + gpsimd queues), a
scalar ACT chain (exp/ln softplus over the gathered block and the full obj
map, one activation-table set), and a short DVE chain computing focal and
smooth-L1 partials with free-axis accumulation.  Output is a [128, 7]
per-partition stats tile; the host reduces the 8x128 rows in float64.

Identities used (bce = BCEWithLogits):
  bce(x, 0) = softplus(x);  bce(x, 1) = softplus(x) - x
  focal (1-pt)^2 = (p-y)^2; 1-p = exp(-softplus(x)); sigmoid = exp(-softplus(-x))
  smooth_l1(d) = 0.5 d^2 - 0.5 relu(d-1)^2 - 0.5 relu(-d-1)^2
  sum softplus(obj)*bg = sum_all softplus - sum_targets softplus(obj_t)/count_t
with count_t (duplicate cells) and unique-cell counts computed on host.
"""

import sys

for _p in ("/opt/trn_rl_repo", "/root/.axon_site/_ro/trn_rl_repo"):
    if _p not in sys.path:
        sys.path.append(_p)

import numpy as np

import concourse.bass as bass
from concourse import mybir
from concourse.bass_utils import run_bass_kernel_spmd

AF = mybir.ActivationFunctionType
ALU = mybir.AluOpType
AX = mybir.AxisListType
F32 = mybir.dt.float32

ALPHA = 0.25
BBOX_W, OBJ_W, CLS_W = 2.0, 1.0, 0.5

M = 8          # cores
B, T, NC_CLS = 16, 32, 63
H3 = W3 = 128
H4 = W4 = 64
BL = B // M    # images per core
N3 = BL * H3 * W3   # 32768 scale3 cells per core
N4 = BL * H4 * W4   # 8192 scale4 cells per core
C3 = N3 // 128      # 256 obj cols of scale3
OBJW = (N3 + N4) // 128  # 320
NT = 2 * BL * T     # 128 targets per core (rows 0:64 scale3, 64:128 scale4)

# gath column layout
G_NOBJ = 0           # -obj_g  (bias for s1)
G_RCNT = 1           # 1/count
G_K4 = 2             # [g2x, g2y, -tw, -th]
G_XY = 6             # cls*onehot (63)
G_YM1 = 69           # onehot-1 (63)
G_GA = 132           # [obj, cls63, -r0, -r1] (66) -> overwritten by softplus
G_NCL = 198          # -clip(r2), -clip(r3)
G_W = 200

_NC_CACHE = None


def _build_bass():
    nc = bass.Bass("TRN2", target_bir_lowering=False, debug=False, num_devices=M)
    gath = nc.declare_dram_parameter("gath", [NT, G_W], F32, isOutput=False)
    objd = nc.declare_dram_parameter("objd", [128, OBJW], F32, isOutput=False)
    outd = nc.declare_dram_parameter("outp", [NT, 7], F32, isOutput=True)

    from contextlib import ExitStack
    with ExitStack() as st:
        def sb(name, shape, dt=F32):
            return st.enter_context(nc.sbuf_tensor(name, shape, dt))

        GT = sb("GT", [NT, G_W])
        E66 = sb("E66", [NT, 66])
        RX = sb("RX", [NT, 68])
        OBJ = sb("OBJ", [128, OBJW])
        EO = sb("EO", [128, OBJW])
        SP = sb("SP", [128, OBJW])
        BCE = sb("BCE", [NT, NC_CLS])
        U1 = sb("U1", [NT, NC_CLS])
        Q2 = sb("Q2", [NT, NC_CLS])
        FQ = sb("FQ", [NT, NC_CLS])
        AC = sb("AC", [NT, 4])
        DT = sb("DT", [NT, 4])
        SQ = sb("SQ", [NT, 4])
        MM = sb("MM", [NT, 8])
        MS = sb("MS", [NT, 8])
        ST = sb("ST", [NT, 7])
        WT = sb("WT", [128, 1])

        g_sem = st.enter_context(nc.semaphore("g_sem"))
        o_sem = st.enter_context(nc.semaphore("o_sem"))
        a_sem = st.enter_context(nc.semaphore("a_sem"))
        d_sem = st.enter_context(nc.semaphore("d_sem"))
        st_sem = st.enter_context(nc.semaphore("st_sem"))
        block = st.enter_context(nc.Block())

        # scl (softplus of GA) overwrites GT[:, G_GA:G_GA+66] in place
        scl0 = GT[:, G_GA:G_GA + 1]          # softplus(obj_g)
        sclx = GT[:, G_GA + 1:G_GA + 64]     # softplus(cls)

        @block.sync
        def _(sync):
            sync.dma_start(out=GT[:], in_=gath[:]).then_inc(g_sem, 16)
            sync.wait_ge(a_sem, 8)
            sync.wait_ge(d_sem, 12)
            sync.dma_start(out=outd[:], in_=ST[:]).then_inc(st_sem, 16)

        @block.gpsimd
        def _(gpsimd):
            gpsimd.dma_start(out=OBJ[:], in_=objd[:]).then_inc(o_sem, 16)

        @block.tensor
        def _(tensor):
            pass

        @block.scalar
        def _(scalar):
            act = nc.scalar
            # warmup: load the exp/ln ACT table before data arrives
            act.activation(out=WT[:], in_=WT[:],
                           func=AF.Exp).then_inc(a_sem, 1)                  # 1
            scalar.wait_ge(g_sem, 16)
            act.activation(out=E66[:], in_=GT[:, G_GA:G_GA + 66],
                           func=AF.Exp).then_inc(a_sem, 1)                  # 2
            act.activation(out=GT[:, G_GA:G_GA + 66], in_=E66[:],
                           func=AF.Ln, bias=1.0).then_inc(a_sem, 1)         # 3
            act.activation(out=RX[:], in_=GT[:, G_GA:G_GA + 68],
                           func=AF.Exp, scale=-1.0).then_inc(a_sem, 1)      # 4
            scalar.wait_ge(o_sem, 16)
            act.activation(out=EO[:], in_=OBJ[:],
                           func=AF.Exp).then_inc(a_sem, 1)                  # 5
            act.activation(out=SP[:], in_=EO[:], func=AF.Ln, bias=1.0,
                           accum_out=ST[:, 5:6]).then_inc(a_sem, 1)         # 6
            act.activation(out=ST[:, 2:3], in_=scl0, func=AF.Identity,
                           bias=GT[:, G_NOBJ:G_NOBJ + 1]).then_inc(a_sem, 1)  # 7
            act.activation(out=ST[:, 4:5], in_=scl0, func=AF.Copy,
                           scale=GT[:, G_RCNT:G_RCNT + 1]).then_inc(a_sem, 1)  # 8

        @block.vector
        def _(vector):
            vec = nc.vector
            vector.wait_ge(a_sem, 3)
            vec.tensor_tensor(out=BCE[:], in0=sclx, in1=GT[:, G_XY:G_XY + 63],
                              op=ALU.subtract).then_inc(d_sem, 1)           # 1
            vector.wait_ge(a_sem, 4)
            vec.tensor_tensor(out=U1[:], in0=RX[:, 1:64],
                              in1=GT[:, G_YM1:G_YM1 + 63],
                              op=ALU.add).then_inc(d_sem, 1)                # 2
            vec.tensor_tensor(out=AC[:], in0=RX[:, 64:68],
                              in1=GT[:, G_K4:G_K4 + 4],
                              op=ALU.add).then_inc(d_sem, 1)                # 3
            nc.vector.drain()
            vec.tensor_tensor(out=Q2[:], in0=U1[:], in1=U1[:],
                              op=ALU.mult).then_inc(d_sem, 1)               # 4
            vec.scalar_tensor_tensor(out=DT[:, 0:2], in0=AC[:, 2:4],
                                     scalar=-0.5, in1=AC[:, 0:2],
                                     op0=ALU.mult,
                                     op1=ALU.add).then_inc(d_sem, 1)        # 5
            vec.scalar_tensor_tensor(out=DT[:, 2:4], in0=AC[:, 2:4],
                                     scalar=0.5, in1=AC[:, 0:2],
                                     op0=ALU.mult,
                                     op1=ALU.add).then_inc(d_sem, 1)        # 6
            nc.vector.drain()
            vec.scalar_tensor_tensor(out=FQ[:], in0=Q2[:], scalar=ALPHA / 63.0,
                                     in1=BCE[:], op0=ALU.mult, op1=ALU.mult,
                                     accum_out=ST[:, 3:4]).then_inc(d_sem, 1)  # 7
            vec.scalar_tensor_tensor(out=SQ[:], in0=DT[:], scalar=1.0,
                                     in1=DT[:], op0=ALU.mult, op1=ALU.mult,
                                     accum_out=ST[:, 0:1]).then_inc(d_sem, 1)  # 8
            vec.tensor_scalar(out=MM[:, 0:4], in0=DT[:], scalar1=1.0,
                              scalar2=-1.0, op0=ALU.max,
                              op1=ALU.add).then_inc(d_sem, 1)               # 9
            vec.tensor_scalar(out=MM[:, 4:8], in0=DT[:], scalar1=-1.0,
                              scalar2=1.0, op0=ALU.min,
                              op1=ALU.add).then_inc(d_sem, 1)               # 10
            nc.vector.drain()
            vec.scalar_tensor_tensor(out=MS[:], in0=MM[:], scalar=1.0,
                                     in1=MM[:], op0=ALU.mult, op1=ALU.mult,
                                     accum_out=ST[:, 1:2]).then_inc(d_sem, 1)  # 11
            vector.wait_ge(a_sem, 6)
            vec.reduce_sum(out=ST[:, 6:7], in_=SP[:, C3:OBJW],
                           axis=AX.X).then_inc(d_sem, 1)                    # 12

    return nc


def _get_bass():
    global _NC_CACHE
    if _NC_CACHE is None:
        _NC_CACHE = _build_bass()
    return _NC_CACHE


def _prep_core_inputs(cls_p3, reg_p3, cls_p4, reg_p4, t3, t4):
    """Slice + gather full inputs into the per-core input maps."""
    f = np.float32
    in_maps = []
    for c in range(M):
        sl = slice(c * BL, (c + 1) * BL)
        gath = np.zeros((NT, G_W), f)
        objs = []
        for s, (cp, rp, lt, hh, ww) in enumerate([
                (cls_p3[sl], reg_p3[sl], t3[sl], H3, W3),
                (cls_p4[sl], reg_p4[sl], t4[sl], H4, W4)]):
            rows = slice(s * BL * T, (s + 1) * BL * T)
            tx = (lt[..., 1] * ww).astype(f)
            ty = (lt[..., 2] * hh).astype(f)
            tw = (lt[..., 3] * ww).astype(f)
            th = (lt[..., 4] * hh).astype(f)
            gx = np.clip(tx, 0, ww - 1).astype(np.int32)
            gy = np.clip(ty, 0, hh - 1).astype(np.int32)
            cid = lt[..., 0].astype(np.int32)
            bb = np.arange(BL)[:, None]

            cls_g = cp[bb, :, gy, gx].astype(f)     # [BL,T,64]
            reg_g = rp[bb, :, gy, gx].astype(f)     # [BL,T,4]
            obj_g = cls_g[..., 0]

            # duplicate-cell counts per (image, cell)
            key = (bb * (hh * ww) + gy * ww + gx).reshape(-1)
            _, inv, cnt = np.unique(key, return_inverse=True,
                                    return_counts=True)
            rcnt = (1.0 / cnt[inv]).astype(f).reshape(BL, T)

            onehot = (np.arange(NC_CLS)[None, None, :]
                      == cid[..., None]).astype(f)

            g = np.zeros((BL * T, G_W), f)
            g[:, G_NOBJ] = (-obj_g).reshape(-1)
            g[:, G_RCNT] = rcnt.reshape(-1)
            g[:, G_K4 + 0] = (gx - tx).reshape(-1)
            g[:, G_K4 + 1] = (gy - ty).reshape(-1)
            g[:, G_K4 + 2] = (-tw).reshape(-1)
            g[:, G_K4 + 3] = (-th).reshape(-1)
            g[:, G_XY:G_XY + 63] = (cls_g[..., 1:] * onehot).reshape(-1, 63)
            g[:, G_YM1:G_YM1 + 63] = (onehot - 1.0).reshape(-1, 63)
            g[:, G_GA] = obj_g.reshape(-1)
            g[:, G_GA + 1:G_GA + 64] = cls_g[..., 1:].reshape(-1, 63)
            g[:, G_GA + 64:G_GA + 66] = (-reg_g[..., 0:2]).reshape(-1, 2)
            g[:, G_NCL:G_NCL + 2] = (
                -np.clip(reg_g[..., 2:4], -4.0, 4.0)).reshape(-1, 2)
            gath[rows] = g
            objs.append(cp[:, 0].reshape(-1))

        obj = np.concatenate(objs).reshape(128, OBJW)
        in_maps.append({
            "gath": np.ascontiguousarray(gath),
            "objd": np.ascontiguousarray(obj, f),
        })
    return in_maps


def _uniq_cells(t, hh, ww):
    tx = t[..., 1] * ww
    ty = t[..., 2] * hh
    gx = np.clip(tx, 0, ww - 1).astype(np.int64)
    gy = np.clip(ty, 0, hh - 1).astype(np.int64)
    bb = np.broadcast_to(np.arange(t.shape[0])[:, None], gx.shape)
    key = bb * (hh * ww) + gy * ww + gx
    return len(np.unique(key))


def _combine(parts, uniq3, uniq4):
    """parts: [M, 128, 7] per-core stats -> scalar loss (float64 combine)."""
    P = np.asarray(parts, np.float64)
    # cols: 0 sum dt^2, 1 sum m^2, 2 obj-pos bce, 3 focal, 4 spo*rcnt,
    #       5 softplus sum (all cells), 6 softplus sum (scale4 cells)
    lb_total = (P[:, :, 0].sum() - P[:, :, 1].sum()) / 8.0
    lo_pos = P[:, :, 2].sum()
    lc_total = P[:, :, 3].sum()
    corr3 = P[:, 0:64, 4].sum()
    corr4 = P[:, 64:128, 4].sum()
    s_tot = P[:, :, 5].sum()
    s4 = P[:, :, 6].sum()
    s3 = s_tot - s4

    bg3 = (s3 - corr3) / max(B * H3 * W3 - uniq3, 1.0)
    bg4 = (s4 - corr4) / max(B * H4 * W4 - uniq4, 1.0)
    n = 2 * B * T
    lb = lb_total / n
    lc = lc_total / n
    lo = (lo_pos + 0.05 * (bg3 + bg4)) / max(n, 1)
    return np.float32(BBOX_W * lb + OBJ_W * lo + CLS_W * lc)


def kernel(cls_p3, reg_p3, cls_p4, reg_p4, t3, t4, _trace=False):
    cls_p3, reg_p3 = np.asarray(cls_p3), np.asarray(reg_p3)
    cls_p4, reg_p4 = np.asarray(cls_p4), np.asarray(reg_p4)
    t3, t4 = np.asarray(t3), np.asarray(t4)
    in_maps = _prep_core_inputs(cls_p3, reg_p3, cls_p4, reg_p4, t3, t4)
    uniq3 = _uniq_cells(t3, H3, W3)
    uniq4 = _uniq_cells(t4, H4, W4)
    nc = _get_bass()
    res = run_bass_kernel_spmd(nc, in_maps, core_ids=list(range(M)),
                               trace=_trace)
    parts = np.stack([r["outp"] for r in res.results])
    out = _combine(parts, uniq3, uniq4)
    if _trace:
        return out, res
    return out


if __name__ == "__main__":
    rng = np.random.default_rng(0)
    inputs = {
        "cls_p3": rng.standard_normal((B, 64, H3, W3)).astype(np.float32),
        "reg_p3": rng.standard_normal((B, 4, H3, W3)).astype(np.float32),
        "cls_p4": rng.standard_normal((B, 64, H4, W4)).astype(np.float32),
        "reg_p4": rng.standard_normal((B, 4, H4, W4)).astype(np.float32),
        "t3": rng.random((B, T, 5)).astype(np.float32),
        "t4": rng.random((B, T, 5)).astype(np.float32),
    }
    print(kernel(**inputs))
